# revision 30
# baseline (speedup 1.0000x reference)
"""GQA attention (b=2, s=2048, d=2048, H=16, Hkv=4, depth=128) on 8 trn2 cores.

Sharding: core c = 4*b + j (b in {0,1}, j in {0..3}) handles batch b and
q-heads {2j, 2j+1, 2j+8, 2j+9}.  This model's RoPE rotates the full projected
vector (pairing dim i with i + d/2), so roped q-head h mixes raw column
blocks {h mod 8, (h mod 8) + 8}; the head grouping above makes the Wq column
shard exactly 512 columns with no duplication.  Those q-heads attend kv-heads
{g0, g0+2} (g0 = 0 for j<2 else 1), which likewise pair up under RoPE.
Each core of a pair projects ONE raw k block and ONE v head; the pair swaps
them with a 2-way AllGather, halving the duplicated K/V projection work.
Wo is row-sharded over the 4 local head-dims; the 4 per-batch bf16 partials
are summed on the host (fp32) and bo added.

Device layout is fully transposed (feature dim on partitions): q_r^T, k_r^T
are [depth, s]; logits are computed as l^T = k_r^T.T @ q_r^T so the softmax
free axis is sq and the PV matmul needs no transposes.  v is transposed to
native [s, dv] layout BEFORE the AllGather with one big DMA-transpose, so
each core transposes only its own head.  All matmuls run in bf16 (fp32 PSUM).

Schedule notes (perf):
- Projections run kc-outer / st-inner so one stationary LdWeights serves 4
  moving passes; redundant consecutive LdWeights are NoOp'd post-lowering.
- PSUM is split into a 4x[128,512] pool and a 2x[128,1024] pool (8 banks);
  projection groups, attention (QK double-buffer + o-banks + denominator
  broadcast) and the Wo loop alternate between them so drains always overlap
  the next group's matmuls.
- Softmax denominators: bf16 pair-tree on DVE at [128,1024] width, a single
  all-ones [128,128] matmul does reduce+broadcast in one PE op, reciprocal
  on DVE (f32), normalize on DVE.  No Ln/Exp round trip on ACT.
- Input DMAs are coalesced (single DMA per weight tensor) and interleaved
  (x chunks with wq quarters) on the Sync queue; the KV exchange chain
  (transpose, collective staging) runs on the Scalar queue.
"""
import numpy as np
import ml_dtypes
from contextlib import ExitStack

import concourse.bass as bass
import concourse.mybir as mybir
import concourse.tile as tile
from concourse.bass import ts
from concourse.bass_utils import run_bass_kernel_spmd

BF = mybir.dt.bfloat16
F32 = mybir.dt.float32
NPBF = ml_dtypes.bfloat16

S = 2048          # sequence length
D = 2048          # d_model
DEPTH = 128       # head dim
NKC = 16          # contraction chunks of 128 over d_model
NST = 4           # 512-wide s tiles
INV_SQRT_D = 1.0 / float(np.sqrt(np.float32(DEPTH)))

_NC_CACHE = None
LAST_RESULT = None  # BassKernelResults of the most recent run (for profiling)


def _split_waits(nc, limit=1):
    """walrus rejects instructions carrying more than a couple of sem waits
    ('Too many sync wait commands').  Move excess waits onto dedicated NoOps
    on the same engine, placed immediately before the instruction."""
    idx = 0
    for f in nc.m.functions:
        for blk in f.blocks:
            insts = blk.instructions
            out = []
            for inst in insts:
                si = inst.sync_info
                if si is not None and len(si.on_wait) > limit:
                    waits = list(si.on_wait)
                    extra, keep = waits[:-limit], waits[-limit:]
                    for w in extra:
                        nop = mybir.InstNoOp(name=f"waitsplit_{idx}", ins=[], outs=[])
                        idx += 1
                        nop.engine = inst.engine
                        nop.bass_nofuse = True
                        nop.sync_info = mybir.SyncInfo(on_wait=[w], on_update=[])
                        out.append(nop)
                    inst.sync_info = mybir.SyncInfo(
                        on_wait=keep, on_update=list(si.on_update)
                    )
                out.append(inst)
            insts[:] = out


def _ap_sig(arg):
    """Signature of a lowered AP argument for LDW dedup."""
    try:
        t = arg.tensor_name if hasattr(arg, "tensor_name") else getattr(arg, "name", None)
        return (str(t), str(getattr(arg, "offset", None)), str(getattr(arg, "ap", None)),
                str(getattr(arg, "dtype", None)))
    except Exception:
        return None


def _dedup_ldweights(nc):
    """Replace InstLdweights that reload the exact same stationary operand
    (with only Matmults in between on PE) with NoOps carrying the same name,
    waits and updates."""
    n_dedup = 0
    for f in nc.m.functions:
        for blk in f.blocks:
            insts = blk.instructions
            last_sig = None
            for idx, inst in enumerate(insts):
                eng = str(inst.engine)
                if not eng.endswith("PE"):
                    continue
                nm = type(inst).__name__
                if nm == "InstLdweights":
                    if getattr(inst, "is_transpose", None):
                        last_sig = None
                        continue
                    sig = _ap_sig(inst.ins[0]) if inst.ins else None
                    if sig is not None and sig == last_sig:
                        nop = mybir.InstNoOp(name=inst.name, ins=[], outs=[])
                        nop.engine = inst.engine
                        nop.bass_nofuse = True
                        if inst.sync_info is not None:
                            nop.sync_info = mybir.SyncInfo(
                                on_wait=list(inst.sync_info.on_wait),
                                on_update=list(inst.sync_info.on_update),
                            )
                        try:
                            nop.set_dependency_edges(inst.dependency_edges)
                        except Exception:
                            pass
                        insts[idx] = nop
                        n_dedup += 1
                    else:
                        last_sig = sig
                elif nm == "InstMatmult":
                    if getattr(inst, "is_transpose", None):
                        last_sig = None
                    continue
                else:
                    last_sig = None
    return n_dedup


def _build_nc():
    nc = bass.Bass(num_devices=8)
    xT = nc.dram_tensor("xT", [128, NKC, S], BF, kind="ExternalInput")
    wq = nc.dram_tensor("wq", [128, NKC, 512], BF, kind="ExternalInput")
    wk = nc.dram_tensor("wk", [128, NKC, 128], BF, kind="ExternalInput")
    wv = nc.dram_tensor("wv", [128, NKC, 128], BF, kind="ExternalInput")
    wo = nc.dram_tensor("wo", [128, 4, D], BF, kind="ExternalInput")
    cq = nc.dram_tensor("cq", [128, 2, S], BF, kind="ExternalInput")
    sq = nc.dram_tensor("sq", [128, 2, S], BF, kind="ExternalInput")
    ck = nc.dram_tensor("ck", [128, S], BF, kind="ExternalInput")
    sk = nc.dram_tensor("sk", [128, S], BF, kind="ExternalInput")
    out = nc.dram_tensor("out", [128, 16, D], BF, kind="ExternalOutput")

    with tile.TileContext(nc) as tc, ExitStack() as top:
        pool_p = top.enter_context(tc.tile_pool(name="persist", bufs=1))
        # 8 PSUM banks: 4x[128,512] + 2x[128,1024]
        po = top.enter_context(tc.tile_pool(name="po", bufs=4, space="PSUM"))
        pp2 = top.enter_context(tc.tile_pool(name="pp2", bufs=2, space="PSUM"))

        qr = pool_p.tile([128, 4, S], BF, tag="qr")   # roped qT [a0,a1,a0+8,a1+8]
        kr = pool_p.tile([128, 2, S], BF, tag="kr")   # roped kT [g0, g0+2]
        vn = pool_p.tile([128, 2, NKC, DEPTH], BF, tag="vn")  # v native
        ones_mat = pool_p.tile([128, 128], BF, tag="ones")
        nc.vector.memset(ones_mat[:], 1.0)

        # pool scopes: p1e closes after the KV exchange, p1m after the last
        # Q projection pass, p2/p3 carry attention + output projection.
        p1m = ExitStack()
        pool_x = p1m.enter_context(tc.tile_pool(name="p1x", bufs=16))
        pool_w = p1m.enter_context(tc.tile_pool(name="p1w", bufs=1))
        pool_tab = p1m.enter_context(tc.tile_pool(name="p1t", bufs=1))
        pool_raw = p1m.enter_context(tc.tile_pool(name="p1raw", bufs=1))
        pool_tmp = p1m.enter_context(tc.tile_pool(name="p1tmp", bufs=2))
        p1e = ExitStack()
        pool_kvw = p1e.enter_context(tc.tile_pool(name="p1kw", bufs=1))
        pool_kv = p1e.enter_context(tc.tile_pool(name="p1kv", bufs=1))
        pool_dram = p1e.enter_context(tc.tile_pool(name="p1dram", bufs=1, space="DRAM"))

        # input stream on the Sync queue: wk/x0/wv first (needed at kc 0),
        # wq after x8 (first needed by pass 2 at ~35us).
        wk_sb = pool_kvw.tile([128, NKC, 128], BF, tag="wk")
        wv_sb = pool_kvw.tile([128, NKC, 128], BF, tag="wv")
        ck_sb = pool_kvw.tile([128, S], BF, tag="ck")
        sk_sb = pool_kvw.tile([128, S], BF, tag="sk")
        wq_sb = pool_w.tile([128, NKC, 512], BF, tag="wq")
        cq_sb = pool_tab.tile([128, 2, S], BF, tag="cq")
        sq_sb = pool_tab.tile([128, 2, S], BF, tag="sq")
        xTs = [pool_x.tile([128, S], BF, tag="xt", name=f"xt_{kc}")
               for kc in range(NKC)]
        nc.sync.dma_start(wk_sb[:], wk[:])
        nc.sync.dma_start(xTs[0][:], xT[:, 0, :])
        nc.sync.dma_start(wv_sb[:], wv[:])
        for kc in range(1, NKC):
            nc.sync.dma_start(xTs[kc][:], xT[:, kc, :])
        for q in range(4):
            nc.sync.dma_start(wq_sb[:, ts(q, 4), :], wq[:, ts(q, 4), :])
        nc.sync.dma_start(cq_sb[:], cq[:])
        nc.sync.dma_start(sq_sb[:], sq[:])
        nc.sync.dma_start(ck_sb[:], ck[:])
        nc.sync.dma_start(sk_sb[:], sk[:])

        # --- pass 1: K + V projections interleaved per kc (PE-efficient;
        # paced by the x-chunk DMA stream at the start) ---
        acc_k = [pp2.tile([128, 1024], F32, tag="ps2", name=f"ak_{i}")
                 for i in range(2)]
        acc_v = [po.tile([128, 512], F32, tag="ps", name=f"av_{st}")
                 for st in range(NST)]
        for kc in range(NKC):
            st_sl = [xTs[kc][:, ts(st, 512)] for st in range(NST)]
            for st in range(NST):
                nc.tensor.matmul(acc_k[st // 2][:, ts(st % 2, 512)],
                                 wk_sb[:, kc, :], st_sl[st],
                                 start=(kc == 0), stop=(kc == NKC - 1))
            for st in range(NST):
                nc.tensor.matmul(acc_v[st][:], wv_sb[:, kc, :], st_sl[st],
                                 start=(kc == 0), stop=(kc == NKC - 1))

        # drains, v-transpose, ONE merged AllGather (raw k | v-native).
        # DMA issues ride the Sync queue (free after the input stream) so the
        # ACT queue never blocks on ring availability; kboth/vn reads park on
        # the AG semaphore at the back of the Sync queue.
        kt_sb = pool_kv.tile([128, S], BF, tag="kt")
        vt_sb = pool_kv.tile([128, S], BF, tag="vt")
        vnat = pool_kv.tile([128, NKC, DEPTH], BF, tag="vnat")
        kboth = pool_kv.tile([128, 2, S], BF, tag="kboth")
        kv_in = pool_dram.tile([128, 2 * S], BF, name="kv_in", tag="ki")
        kv_out = pool_dram.tile([2, 128, 2 * S], BF, name="kv_out", tag="ko")
        for i in range(2):
            nc.scalar.copy(kt_sb[:, ts(i, 1024)], acc_k[i][:])
        for st in range(NST):
            nc.scalar.copy(vt_sb[:, ts(st, 512)], acc_v[st][:])
        nc.sync.dma_start(kv_in[:, 0:S], kt_sb[:])
        nc.sync.dma_start_transpose(vnat[:], vt_sb[:])
        nc.sync.dma_start(kv_in[:, S:2 * S], vnat[:])
        nc.gpsimd.collective_compute(
            "AllGather",
            mybir.AluOpType.bypass,
            replica_groups=[[0, 1], [2, 3], [4, 5], [6, 7]],
            ins=[kv_in.opt()],
            outs=[kv_out.opt()],
        )
        nc.sync.dma_start(vn[:, 0, :, :], kv_out[0, :, S:2 * S])
        for r in range(2):
            nc.sync.dma_start(kboth[:, r, :], kv_out[r, :, 0:S])
        nc.sync.dma_start(vn[:, 1, :, :], kv_out[1, :, S:2 * S])

        # --- Q projections, split into per-raw-block sub-passes so they can
        # interleave with the attention schedule ---
        qraws = {}

        def qsub(i, xb, use_pp2):
            """Project raw block (i, xb) of Wq over all of x into a bf16 raw
            tile (64 matmuls kc-outer, one stationary per kc)."""
            blk = i + 2 * xb
            raw = pool_raw.tile([128, S], BF, tag=f"raw{xb}",
                                name=f"qraw_{i}_{xb}")
            qraws[(i, xb)] = raw
            if use_pp2:
                accs = [pp2.tile([128, 1024], F32, tag="ps2",
                                 name=f"aq2_{i}_{xb}_{h}") for h in range(2)]
                for kc in range(NKC):
                    for st in range(NST):
                        nc.tensor.matmul(accs[st // 2][:, ts(st % 2, 512)],
                                         wq_sb[:, kc, ts(blk, 128)],
                                         xTs[kc][:, ts(st, 512)],
                                         start=(kc == 0), stop=(kc == NKC - 1))
                for st in range(NST):
                    nc.scalar.copy(raw[:, ts(st, 512)],
                                   accs[st // 2][:, ts(st % 2, 512)])
            else:
                accs = [po.tile([128, 512], F32, tag="ps",
                                name=f"aq_{i}_{xb}_{st}") for st in range(NST)]
                for kc in range(NKC):
                    for st in range(NST):
                        nc.tensor.matmul(accs[st][:],
                                         wq_sb[:, kc, ts(blk, 128)],
                                         xTs[kc][:, ts(st, 512)],
                                         start=(kc == 0), stop=(kc == NKC - 1))
                for st in range(NST):
                    nc.scalar.copy(raw[:, ts(st, 512)], accs[st][:])

        def qrope(i):
            raw0, raw1 = qraws[(i, 0)], qraws[(i, 1)]
            for st in range(NST):
                sl = ts(st, 512)
                x1, x2 = raw0[:, sl], raw1[:, sl]
                c_ap, s_ap = cq_sb[:, i, sl], sq_sb[:, i, sl]
                t1 = pool_tmp.tile([128, 512], BF, tag="t")
                t2 = pool_tmp.tile([128, 512], BF, tag="t")
                nc.vector.tensor_mul(t1[:], x1, c_ap)
                nc.vector.tensor_mul(t2[:], x2, s_ap)
                nc.vector.tensor_sub(qr[:, i, sl], t1[:], t2[:])
                t3 = pool_tmp.tile([128, 512], BF, tag="t")
                t4 = pool_tmp.tile([128, 512], BF, tag="t")
                nc.vector.tensor_mul(t3[:], x2, c_ap)
                nc.vector.tensor_mul(t4[:], x1, s_ap)
                nc.vector.tensor_add(qr[:, 2 + i, sl], t3[:], t4[:])

        qsub(0, 0, use_pp2=True)
        qsub(0, 1, use_pp2=False)
        qrope(0)
        # head-pair 1 also projects BEFORE the attends: its 28us of matmuls
        # cover the AllGather protocol+mesh latency so attention starts with
        # kr/vn already resident.

        # k rope between qrope(0) and qrope(1) on the DVE queue: it waits on
        # the AllGather and attend(0,0) consumes its st chunks just-in-time;
        # qrope(1) (needed ~40us later) queues behind it.  (x1 = even core's raw block g0, x2 = block g0+2)
        for st in range(NST):
            sl = ts(st, 512)
            x1, x2 = kboth[:, 0, sl], kboth[:, 1, sl]
            c_ap, s_ap = ck_sb[:, sl], sk_sb[:, sl]
            t1 = pool_tmp.tile([128, 512], BF, tag="t")
            t2 = pool_tmp.tile([128, 512], BF, tag="t")
            nc.vector.tensor_mul(t1[:], x1, c_ap)
            nc.vector.tensor_mul(t2[:], x2, s_ap)
            nc.vector.tensor_sub(kr[:, 0, sl], t1[:], t2[:])
            t3 = pool_tmp.tile([128, 512], BF, tag="t")
            t4 = pool_tmp.tile([128, 512], BF, tag="t")
            nc.vector.tensor_mul(t3[:], x2, c_ap)
            nc.vector.tensor_mul(t4[:], x1, s_ap)
            nc.vector.tensor_add(kr[:, 1, sl], t3[:], t4[:])
        qsub(1, 0, use_pp2=False)
        qsub(1, 1, use_pp2=True)
        qrope(1)
        p1e.close()

        # ---------------- attention, half-major, Wo interleaved ----------
        p2 = ExitStack()
        pool_exp = p2.enter_context(tc.tile_pool(name="exp", bufs=8))
        pool_pair = p2.enter_context(tc.tile_pool(name="pair", bufs=3))
        pool_sum = p2.enter_context(tc.tile_pool(name="sum", bufs=2))
        pool_rec = p2.enter_context(tc.tile_pool(name="rec", bufs=1))
        # onorm aliases qr: attend(hi, half) finishes reading qr[:, hi, cols]
        # exactly before the (deferred) normalize writes those columns.
        onorm = qr

        def attend_half(hi, half):
            g = hi // 2
            sts = (2 * half, 2 * half + 1)
            o_banks = [po.tile([128, 512], F32, tag="ps",
                               name=f"ob_{hi}_{st}") for st in sts]
            sums = pool_sum.tile([128, 1024], BF, tag="sums",
                                 name=f"sum_{hi}_{half}")
            pairs = [pool_pair.tile([128, 1024], BF, tag="pair",
                                    name=f"pr_{hi}_{half}_{k}")
                     for k in range(NKC // 2)]
            # software-pipelined by one skt: QK(s+1) is EMITTED before
            # exp(s)/PV(s), so the PE never gates the ACT exp stream and sem
            # jitter is absorbed by the ready-issued QK.
            prev = None
            lgs = {}
            es = {}

            def stage_qk(skt):
                lg2 = pp2.tile([128, 1024], F32, tag="ps2",
                               name=f"lg_{hi}_{half}_{skt}")
                lgs[skt] = lg2
                es[skt] = pool_exp.tile([128, 1024], BF, tag="exp",
                                        name=f"e_{hi}_{half}_{skt}")
                for idx, st in enumerate(sts):
                    nc.tensor.matmul(
                        lg2[:, ts(idx, 512)],
                        kr[:, g, ts(skt, 128)],
                        qr[:, hi, ts(st, 512)],
                        start=True, stop=True,
                    )

            def stage_consume(skt):
                nonlocal prev
                e = es[skt]
                nc.scalar.activation(
                    e[:], lgs.pop(skt)[:],
                    mybir.ActivationFunctionType.Exp,
                    scale=INV_SQRT_D,
                )
                if skt % 2 == 0:
                    prev = e
                else:
                    k = skt // 2
                    nc.vector.tensor_add(pairs[k][:], prev[:], e[:])
                    if k == 1:
                        nc.vector.tensor_add(sums[:], pairs[0][:], pairs[1][:])
                    elif k > 1:
                        nc.vector.tensor_add(sums[:], sums[:], pairs[k][:])
                for idx, st in enumerate(sts):
                    nc.tensor.matmul(
                        o_banks[idx][:],
                        vn[:, g, skt, :],
                        e[:, ts(idx, 512)],
                        start=(skt == 0),
                        stop=(skt == NKC - 1),
                    )

            stage_qk(0)
            for skt in range(1, NKC):
                stage_qk(skt)
                stage_consume(skt - 1)
            stage_consume(NKC - 1)

            # denominator chain inline at the END of the half — the only
            # pp2-ring position where the bc allocation lands on a consumed
            # lg slot under the pipelined qk order.  Single ones-matmul does
            # reduce+broadcast per st, 1/d = exp(-ln(d)) on ACT, normalize
            # on DVE; the next half's issued-ahead QKs cover the ACT bubble.
            bc = pp2.tile([128, 1024], F32, tag="ps2", name=f"bc_{hi}_{half}")
            for idx in range(2):
                nc.tensor.matmul(bc[:, ts(idx, 512)], ones_mat[:],
                                 sums[:, ts(idx, 512)],
                                 start=True, stop=True)
            lbc = pool_rec.tile([128, 1024], F32, tag="lbc")
            nc.scalar.activation(lbc[:], bc[:],
                                 mybir.ActivationFunctionType.Ln)
            rec = pool_rec.tile([128, 1024], F32, tag="rec")
            nc.scalar.activation(rec[:], lbc[:],
                                 mybir.ActivationFunctionType.Exp,
                                 scale=-1.0)
            for idx, st in enumerate(sts):
                nc.vector.tensor_mul(
                    onorm[:, hi, ts(st, 512)], o_banks[idx][:],
                    rec[:, ts(idx, 512)]
                )

        def wo_tile(m, dve_only):
            if m % 2 == 0:
                obanks = [po.tile([128, 512], F32, tag="ps",
                                  name=f"op_{m}_{i}")[:] for i in range(4)]
                ob2 = None
            else:
                ob2 = [pp2.tile([128, 1024], F32, tag="ps2",
                                name=f"op_{m}_{i}") for i in range(2)]
                obanks = [ob2[ct // 2][:, ts(ct % 2, 512)] for ct in range(4)]
            for hi in range(4):
                for ct in range(4):
                    nc.tensor.matmul(
                        obanks[ct],
                        onorm[:, hi, ts(m, 128)],
                        wo_sb[:, hi, ts(ct, 512)],
                        start=(hi == 0),
                        stop=(hi == 3),
                    )
            o_sb = pool_out.tile([128, D], BF, tag="out")
            if m % 2 == 0:
                for ct in range(4):
                    if dve_only or ct % 2 == 0:
                        nc.vector.tensor_copy(o_sb[:, ts(ct, 512)], obanks[ct])
                    else:
                        nc.scalar.copy(o_sb[:, ts(ct, 512)], obanks[ct])
            else:
                nc.vector.tensor_copy(o_sb[:, ts(0, 1024)], ob2[0][:])
                if dve_only:
                    nc.vector.tensor_copy(o_sb[:, ts(1, 1024)], ob2[1][:])
                else:
                    nc.scalar.copy(o_sb[:, ts(1, 1024)], ob2[1][:])
            nc.sync.dma_start(out[:, m, :], o_sb[:])

        # first halves of every head (kv group order g0,g1,g0,g1)
        attend_half(0, 0)
        attend_half(2, 0)

        p3 = ExitStack()
        pool_wo = p3.enter_context(tc.tile_pool(name="wop", bufs=1))
        pool_out = p3.enter_context(tc.tile_pool(name="osb", bufs=2))
        wo_sb = pool_wo.tile([128, 4, D], BF)
        nc.sync.dma_start(wo_sb[:], wo[:])

        attend_half(1, 0)
        attend_half(3, 0)

        # second halves, each followed by its denominator chain and two Wo
        # m-tiles (columns 0..1023 of onorm are complete after the first
        # halves) — the Wo matmuls keep PE busy through the ACT-bound
        # exp/Ln windows.  Drains stay on DVE here (ACT is saturated).
        for w_i, hi in enumerate((0, 2, 1, 3)):
            attend_half(hi, 1)
            wo_tile(2 * w_i + 1, dve_only=True)
            wo_tile(2 * w_i, dve_only=True)
        for m in range(8, 16):
            wo_tile(m, dve_only=False)
        p3.close()
        p2.close()
        p1m.close()

    # NOTE: do NOT run _dedup_ldweights here — walrus assigns PE weight-buffer
    # slots per LdWeights, so removing "redundant" ones corrupts the matmuls
    # (verified on hardware).
    _split_waits(nc)
    return nc


def _chunk128(arr):
    """(K*128, N) f32 -> [128, K, N] bf16 with [p, k, n] = arr[k*128+p, n]."""
    k = arr.shape[0] // 128
    return np.ascontiguousarray(
        arr.reshape(k, 128, arr.shape[1]).transpose(1, 0, 2)
    ).astype(NPBF)


def _rope_tables(dim):
    pos = np.arange(S, dtype=np.float32)
    inv = (10000.0 ** (-(np.arange(dim, dtype=np.float32)) / np.float32(dim))
           ).astype(np.float32)
    freqs = pos[:, None] * inv[None, :]
    return np.cos(freqs).astype(np.float32), np.sin(freqs).astype(np.float32)


def kernel(x, mask, Wq, Wk, Wv, Wo, bo):
    global _NC_CACHE
    assert np.asarray(mask).all(), "kernel specialized for all-true mask"
    x = np.asarray(x, dtype=np.float32)
    Wq = np.asarray(Wq, dtype=np.float32)
    Wk = np.asarray(Wk, dtype=np.float32)
    Wv = np.asarray(Wv, dtype=np.float32)
    Wo = np.asarray(Wo, dtype=np.float32)
    bo = np.asarray(bo, dtype=np.float32)

    cos_q, sin_q = _rope_tables(1024)
    cos_k, sin_k = _rope_tables(256)

    def blk(a, i):  # column block i (width 128) of a
        return a[:, i * 128:(i + 1) * 128]

    in_maps = []
    for c in range(8):
        b, j = c // 4, c % 4
        a0, a1 = 2 * j, 2 * j + 1
        g0 = 0 if j < 2 else 1

        xb = x[b]                                   # (S, D)
        xT3 = _chunk128(np.ascontiguousarray(xb.T))  # [128, 16, S]

        wq_sel = np.concatenate(
            [blk(Wq, a0), blk(Wq, a1), blk(Wq, a0 + 8), blk(Wq, a1 + 8)], axis=1)
        myblk = g0 + 2 * (j % 2)
        wk_sel = blk(Wk, myblk)
        wv_sel = blk(Wv, myblk)
        wo_sel = np.concatenate(
            [Wo[h * 128:(h + 1) * 128, :] for h in (a0, a1, a0 + 8, a1 + 8)],
            axis=0)

        cq_sel = _chunk128(np.ascontiguousarray(
            np.concatenate([blk(cos_q, a0), blk(cos_q, a1)], axis=1).T))
        sq_sel = _chunk128(np.ascontiguousarray(
            np.concatenate([blk(sin_q, a0), blk(sin_q, a1)], axis=1).T))
        ck_sel = np.ascontiguousarray(blk(cos_k, g0).T).astype(NPBF)
        sk_sel = np.ascontiguousarray(blk(sin_k, g0).T).astype(NPBF)

        in_maps.append({
            "xT": xT3,
            "wq": _chunk128(wq_sel),
            "wk": _chunk128(wk_sel),
            "wv": _chunk128(wv_sel),
            "wo": _chunk128(wo_sel),
            "cq": cq_sel, "sq": sq_sel, "ck": ck_sel, "sk": sk_sel,
        })

    global LAST_RESULT
    if _NC_CACHE is None:
        _NC_CACHE = _build_nc()
    res = run_bass_kernel_spmd(_NC_CACHE, in_maps, list(range(8)))
    LAST_RESULT = res

    partials = [
        res.results[c]["out"].astype(np.float32).transpose(1, 0, 2).reshape(S, D)
        for c in range(8)
    ]
    out = np.stack(
        [sum(partials[4 * b + j] for j in range(4)) for b in range(2)], axis=0
    )
    return (out + bo).astype(np.float32)
